# revision 1
# baseline (speedup 1.0000x reference)
"""Trainium2 Bass kernel for the two-template sparse cross-modal attention module.

Sharding: data-parallel over batch B=32 across 8 NeuronCores (4 samples/core).
Each sample carries two modality streams (v, i) that must be co-resident
because search tokens attend to the template keys of BOTH modalities.

Per-core program (per sample s, streams st in {v, i}):
  1. QK^T projection in transposed layout: QKT[1536, 384] = qkv_w[0:1536] @ x.T
     (lhsT = qkv_w.T chunks, rhs = x.T chunks) -> per-head Q.T, K.T [64, tok].
  2. V projection in natural layout: V[384, 768] = x @ qkv_w[1536:].T
     (lhsT = x.T chunks, rhs = qkv_w.T[:, 1536:]) stored with a ones column
     per head ([tok, 65]) so the AV matmul also accumulates the softmax
     denominator l as an extra output row.
  3. Attention per head, scores transposed (S.T[k, q] = K Q.T, contract Dh):
     softmax without max-subtraction (scores are O(1); exp is safe), the
     denominator comes from the ones column, normalization by 1/l applied via
     a gpsimd partition_broadcast of recip_l + one DVE multiply.
     Search queries attend to [own k_mt, other-modality k_mt, own k_s].
  4. Output projection from the transposed attention output (lhsT = O.T
     chunks, rhs = proj_w.T) -> natural-layout Y [384, 768], bias added via a
     K=1 ones matmul, contiguous DMA out.
"""

import numpy as np

for _p in ("/opt/trn_rl_repo", "/root/.axon_site/_ro/trn_rl_repo"):
    import os
    import sys

    if os.path.isdir(_p) and _p not in sys.path:
        sys.path.append(_p)

B = 32
N_CORES = 8
SAMPLES = 4  # per core
C = 768
NTOK = 384
H = 12
DH = 64
MT = 128  # template tokens
CCH = C // 128  # 6 contraction chunks
MCH = 12  # QK row chunks (1536/128)
TCH = NTOK // 128  # 3 token chunks
SCALE = DH ** (-0.5)

_PROG_CACHE = {}


def _build_program(mm_f32r, es_bf16, with_bias=True):
    import concourse.bass as bass  # noqa: F401
    import concourse.tile as tile
    from concourse import bacc, mybir

    f32 = mybir.dt.float32
    f32r = mybir.dt.float32r
    bf16 = mybir.dt.bfloat16
    mdt = f32r if mm_f32r else f32
    esdt = bf16 if es_bf16 else mdt
    Act = mybir.ActivationFunctionType

    nc = bacc.Bacc(None, target_bir_lowering=False)
    if mm_f32r or es_bf16:
        import contextlib

        _lp = nc.allow_low_precision(reason="fp32r/bf16 matmul inputs, fp32 PSUM accumulation")
    else:
        import contextlib

        _lp = contextlib.nullcontext()
    _lp.__enter__()

    xt_d = nc.dram_tensor("xt", [2 * SAMPLES, C, NTOK], f32, kind="ExternalInput")
    qkvw_d = nc.dram_tensor("qkvwT", [C, 3 * C], f32, kind="ExternalInput")
    projw_d = nc.dram_tensor("projwT", [C, C], f32, kind="ExternalInput")
    bias_d = nc.dram_tensor("bias", [1, C], f32, kind="ExternalInput")
    y_d = nc.dram_tensor("y", [2 * SAMPLES, NTOK, C], f32, kind="ExternalOutput")

    dma_in = nc.gpsimd if mm_f32r else nc.sync

    with tile.TileContext(nc) as tc:
        with (
            tc.tile_pool(name="consts", bufs=1) as consts,
            tc.tile_pool(name="xtp", bufs=2) as xtp,
            tc.tile_pool(name="qktp", bufs=1) as qktp,
            tc.tile_pool(name="v1p", bufs=1) as v1p,
            tc.tile_pool(name="otp", bufs=1) as otp,
            tc.tile_pool(name="esp", bufs=4) as esp,
            tc.tile_pool(name="rlp", bufs=2) as rlp,
            tc.tile_pool(name="rlbp", bufs=2) as rlbp,
            tc.tile_pool(name="yp", bufs=3) as yp,
            tc.tile_pool(name="pap", bufs=3, space="PSUM") as pap,
            tc.tile_pool(name="psp", bufs=3, space="PSUM") as psp,
            tc.tile_pool(name="pop", bufs=2, space="PSUM") as pop,
        ):
            qkvw_sb = consts.tile([128, CCH, 3 * C], mdt)
            projw_sb = consts.tile([128, CCH, C], mdt)
            bias_sb = consts.tile([1, C], mdt)
            ones_row = consts.tile([1, 128], mdt)
            ones_f32 = consts.tile([128, 128], f32)
            nc.vector.memset(ones_f32, 1.0)
            for c in range(CCH):
                dma_in.dma_start(
                    out=qkvw_sb[:, c, :], in_=qkvw_d[c * 128 : (c + 1) * 128, :]
                )
                dma_in.dma_start(
                    out=projw_sb[:, c, :], in_=projw_d[c * 128 : (c + 1) * 128, :]
                )
            dma_in.dma_start(out=bias_sb, in_=bias_d[:, :])
            nc.vector.tensor_copy(out=ones_row, in_=ones_f32[0:1, 0:128])

            for s in range(SAMPLES):
                xt_sb = xtp.tile([128, CCH, 2, NTOK], mdt, tag="xt")
                for st in range(2):
                    for c in range(CCH):
                        dma_in.dma_start(
                            out=xt_sb[:, c, st, :],
                            in_=xt_d[2 * s + st, c * 128 : (c + 1) * 128, :],
                        )

                # ---- phase 1: QK^T (transposed layout) ----
                qkt_sb = qktp.tile([128, MCH, 2, NTOK], mdt, tag="qkt")
                for m in range(MCH):
                    for st in range(2):
                        pq = pap.tile([128, NTOK], f32, tag="pa")
                        for c in range(CCH):
                            nc.tensor.matmul(
                                pq,
                                qkvw_sb[:, c, m * 128 : (m + 1) * 128],
                                xt_sb[:, c, st, :],
                                start=(c == 0),
                                stop=(c == CCH - 1),
                            )
                        nc.scalar.activation(
                            out=qkt_sb[:, m, st, :], in_=pq, func=Act.Copy
                        )

                # ---- phase 2: V (natural layout, with ones column) ----
                v1_sb = v1p.tile([128, TCH, 2, H, 65], mdt, tag="v1")
                for t in range(TCH):
                    for st in range(2):
                        for n in range(2):
                            pv = pap.tile([128, NTOK], f32, tag="pa")
                            for c in range(CCH):
                                nc.tensor.matmul(
                                    pv,
                                    xt_sb[:, c, st, t * 128 : (t + 1) * 128],
                                    qkvw_sb[:, c, 2 * C + n * NTOK : 2 * C + (n + 1) * NTOK],
                                    start=(c == 0),
                                    stop=(c == CCH - 1),
                                )
                            nc.vector.tensor_copy(
                                out=v1_sb[:, t, st, 6 * n : 6 * n + 6, 0:64],
                                in_=pv.rearrange("p (h d) -> p h d", h=6),
                            )
                nc.vector.tensor_copy(
                    out=v1_sb[:, :, :, :, 64:65],
                    in_=ones_f32[:, 0:72].rearrange(
                        "p (t s h) -> p t s h", t=TCH, s=2
                    ).unsqueeze(4),
                )

                # ---- phase 3: attention ----
                # Heads are processed in even/odd pairs: their Q.T/K.T slices
                # sit at partition bases 0 and 64, so the two K=64 score
                # matmuls target distinct PE row-groups; emitting them
                # back-to-back lets the hardware run them concurrently.
                ot_sb = otp.tile([128, CCH, 2, NTOK], mdt, tag="ot")
                for st in range(2):
                    for hp in range(6):
                        po_pair = [
                            pop.tile([65, NTOK], f32, tag="po", name=f"po_{s}_{st}_{hp}_{i}")
                            for i in range(2)
                        ]
                        # per chunk: S-mm pair (adjacent), exps, AV pair
                        for ci in range(4):
                            es_pair = []
                            ps_pair = []
                            for i in range(2):
                                h = 2 * hp + i
                                ro = i * 64
                                qT = qkt_sb[ro : ro + 64, hp, st, :]
                                kT = qkt_sb[ro : ro + 64, 6 + hp, st, :]
                                kTo = qkt_sb[ro : ro + 64, 6 + hp, 1 - st, :]
                                if ci == 0:
                                    lk, rq, nq = kT[:, 0:MT], qT, NTOK
                                elif ci == 1:
                                    lk, rq, nq = kTo[:, 0:MT], qT[:, MT:], 256
                                else:
                                    j = ci - 2
                                    lk = kT[:, MT + j * 128 : MT + (j + 1) * 128]
                                    rq, nq = qT[:, MT:], 256
                                psc = psp.tile(
                                    [128, nq], f32, tag="ps", name=f"ps_{s}_{st}_{hp}_{ci}_{i}"
                                )
                                nc.tensor.matmul(psc, lk, rq, start=True, stop=True)
                                ps_pair.append(psc)
                            for i in range(2):
                                ei = esp.tile(
                                    [128, nq], esdt, tag="es", name=f"es_{s}_{st}_{hp}_{ci}_{i}"
                                )
                                nc.scalar.activation(
                                    ei, ps_pair[i], Act.Exp, scale=SCALE
                                )
                                es_pair.append(ei)
                            for i in range(2):
                                h = 2 * hp + i
                                vst = (1 - st) if ci == 1 else st
                                vt = 0 if ci < 2 else ci - 1
                                dst = po_pair[i] if ci == 0 else po_pair[i][:, MT:]
                                nc.tensor.matmul(
                                    dst,
                                    v1_sb[:, vt, vst, h, :],
                                    es_pair[i],
                                    start=(ci == 0),
                                    stop=(ci == 3),
                                )
                        for i in range(2):
                            h = 2 * hp + i
                            ro = i * 64
                            po = po_pair[i]
                            rl = rlp.tile([1, NTOK], f32, tag="rl", name=f"rl_{s}_{st}_{hp}_{i}")
                            nc.vector.reciprocal(out=rl, in_=po[64:65, :])
                            rlb = rlbp.tile([64, NTOK], f32, tag="rlb", name=f"rlb_{s}_{st}_{hp}_{i}")
                            nc.gpsimd.partition_broadcast(rlb, rl)
                            nc.vector.tensor_mul(
                                ot_sb[ro : ro + 64, hp, st, :], po[0:64, :], rlb
                            )

                # ---- phase 4: output projection ----
                for st in range(2):
                    for t in range(TCH):
                        y_sb = yp.tile([128, C], f32, tag="y")
                        for n2 in range(2):
                            py = pap.tile([128, NTOK], f32, tag="pa")
                            for c in range(CCH):
                                nc.tensor.matmul(
                                    py,
                                    ot_sb[:, c, st, t * 128 : (t + 1) * 128],
                                    projw_sb[:, c, n2 * NTOK : (n2 + 1) * NTOK],
                                    start=(c == 0),
                                    stop=(not with_bias and c == CCH - 1),
                                )
                            if with_bias:
                                nc.tensor.matmul(
                                    py,
                                    ones_row[0:1, :],
                                    bias_sb[0:1, n2 * NTOK : (n2 + 1) * NTOK],
                                    start=False,
                                    stop=True,
                                )
                            nc.vector.tensor_copy(
                                out=y_sb[:, n2 * NTOK : (n2 + 1) * NTOK], in_=py
                            )
                        nc.sync.dma_start(
                            out=y_d[2 * s + st, t * 128 : (t + 1) * 128, :], in_=y_sb
                        )

    _lp.__exit__(None, None, None)
    nc.compile()
    return nc


def _get_program(mm_f32r=True, es_bf16=False, with_bias=True):
    key = (mm_f32r, es_bf16, with_bias)
    if key not in _PROG_CACHE:
        _PROG_CACHE[key] = _build_program(mm_f32r, es_bf16, with_bias)
    return _PROG_CACHE[key]


def _prep_in_maps(x_v, x_i, qkv_w, proj_w, proj_b):
    qkvwT = np.ascontiguousarray(qkv_w.T.astype(np.float32))
    projwT = np.ascontiguousarray(proj_w.T.astype(np.float32))
    bias = np.ascontiguousarray(proj_b.astype(np.float32).reshape(1, C))
    in_maps = []
    for core in range(N_CORES):
        sl = slice(core * SAMPLES, (core + 1) * SAMPLES)
        # interleave: stream 2s = v-sample, 2s+1 = i-sample, transposed to [C, NTOK]
        xs = np.empty((2 * SAMPLES, C, NTOK), np.float32)
        xs[0::2] = np.asarray(x_v[sl]).transpose(0, 2, 1)
        xs[1::2] = np.asarray(x_i[sl]).transpose(0, 2, 1)
        in_maps.append(
            {
                "xt": np.ascontiguousarray(xs),
                "qkvwT": qkvwT,
                "projwT": projwT,
                "bias": bias,
            }
        )
    return in_maps


def kernel(x_v, x_i, qkv_w, proj_w, proj_b, t_h, t_w, s_h, s_w, num_heads):
    from concourse.bass_utils import run_bass_kernel_spmd

    x_v = np.asarray(x_v, np.float32)
    x_i = np.asarray(x_i, np.float32)
    nc = _get_program(with_bias=bool(np.any(np.asarray(proj_b))))
    in_maps = _prep_in_maps(x_v, x_i, qkv_w, proj_w, proj_b)
    res = run_bass_kernel_spmd(nc, in_maps, list(range(N_CORES)))
    out_v = np.empty((B, NTOK, C), np.float32)
    out_i = np.empty((B, NTOK, C), np.float32)
    for core in range(N_CORES):
        y = res.results[core]["y"]
        sl = slice(core * SAMPLES, (core + 1) * SAMPLES)
        out_v[sl] = y[0::2]
        out_i[sl] = y[1::2]
    return out_v, out_i



# revision 55
# speedup vs baseline: 1.4011x; 1.4011x over previous
"""Trainium2 Bass kernel for the two-template sparse cross-modal attention module.

Sharding: data-parallel over batch B=32 across 8 NeuronCores (4 samples/core).
Each sample carries two modality streams (v, i) that must be co-resident
because search tokens attend to the template keys of BOTH modalities.

All matmul operands are bf16 (converted on the host, so DMA moves half the
bytes and every matmul runs at 1 cycle/row regardless of moving size); PSUM
accumulation stays f32. Every matmul contracts over the full 128 partitions at
tile position (0, 0): per-head K tensors are stored zero-padded on the other
head's 64 partition rows (bf16 K=64 matmuls at mixed tile positions
miscompile), which also lets S-matmuls take the packed two-head Q tile as rhs.

Per-core program (per sample s, streams st in {v, i}):
  ph1  QK^T in transposed layout: QKT[1536, 384] = qkv_w[0:1536] @ x.T.
       Q chunks copy PSUM->SBUF whole; K chunks split per head into the
       zero-padded kpad tiles (parity-alternating across samples).
  ph2  V in natural layout with a ones column per head ([tok, 65]) so the AV
       matmul also accumulates the softmax denominator l as an extra row.
  ph3  Attention per head pair, scores transposed (S.T[k, q] = K Q.T):
       softmax without max-subtraction (scores are O(1)), both heads' template
       scores exp'd in one strided activation, denominator from the ones
       column, normalization via DVE reciprocal + gpsimd partition_broadcast +
       DVE mul. The template-score/exp block for pair p+1 is emitted inside
       pair p so the exp latency stays off the first AV matmul's critical path.
  ph4  Output projection; bias added during the PSUM->SBUF move as a DVE
       tensor_add against a pre-broadcast [128, C] bias tile.

Scheduling: ph1/ph2/ph4 are emitted as independent "filler units" (6-matmul
chain + one move). During ph3, up to 3 filler units per pair are drained at
the points where the PE would otherwise stall on exp->AV dependencies; the
cap makes leftover ph4 units roll forward into the filler-poor final sample.
PSUM->SBUF moves alternate between DVE and the scalar engine. Startup DMAs
are ordered by first use across four engine queues (the DMA engines are a
serial resource in the cost model).
"""

import numpy as np

for _p in ("/opt/trn_rl_repo", "/root/.axon_site/_ro/trn_rl_repo"):
    import os
    import sys

    if os.path.isdir(_p) and _p not in sys.path:
        sys.path.append(_p)

B = 32
N_CORES = 8
SAMPLES = 4  # per core
C = 768
NTOK = 384
H = 12
DH = 64
MT = 128  # template tokens per stream
CCH = C // 128  # 6 contraction chunks
MCH = 12  # QK row chunks (1536/128)
TCH = NTOK // 128  # 3 token chunks
SCALE = DH ** (-0.5)

_PROG_CACHE = {}


def _build_program():
    import concourse.bass as bass  # noqa: F401
    import concourse.tile as tile
    from concourse import bacc, mybir

    f32 = mybir.dt.float32
    f32r = mybir.dt.float32r
    bf16 = mybir.dt.bfloat16
    Act = mybir.ActivationFunctionType

    nc = bacc.Bacc(None, target_bir_lowering=False)
    _lp = nc.allow_low_precision(reason="bf16 matmul inputs, fp32 PSUM accumulation")
    _lp.__enter__()

    xt_d = nc.dram_tensor("xt", [2 * SAMPLES, C, NTOK], bf16, kind="ExternalInput")
    qkvw_d = nc.dram_tensor("qkvwT", [C, 3 * C], bf16, kind="ExternalInput")
    projw_d = nc.dram_tensor("projwT", [C, C], bf16, kind="ExternalInput")
    bias_d = nc.dram_tensor("bias", [1, C], f32, kind="ExternalInput")
    y_d = nc.dram_tensor("y", [2 * SAMPLES, NTOK, C], f32, kind="ExternalOutput")

    # DRAM views with the (chunk, partition) split on the contraction dim.
    qkvw_v = qkvw_d.rearrange("(c p) m -> p c m", p=128)
    projw_v = projw_d.rearrange("(c p) m -> p c m", p=128)

    with tile.TileContext(nc) as tc:
        with (
            tc.tile_pool(name="consts", bufs=1) as consts,
            tc.tile_pool(name="xtp", bufs=3) as xtp,
            tc.tile_pool(name="qktp", bufs=3) as qktp,
            tc.tile_pool(name="v1p", bufs=2) as v1p,
            tc.tile_pool(name="otp", bufs=3) as otp,
            tc.tile_pool(name="es0p", bufs=4) as es0p,
            tc.tile_pool(name="esSp", bufs=4) as esSp,
            tc.tile_pool(name="rlp", bufs=2) as rlp,
            tc.tile_pool(name="rlbp", bufs=2) as rlbp,
            tc.tile_pool(name="yp", bufs=3) as yp,
            tc.tile_pool(name="pap", bufs=3, space="PSUM") as pap,
            tc.tile_pool(name="scSp", bufs=3, space="PSUM") as scSp,
            tc.tile_pool(name="pop", bufs=2, space="PSUM") as pop,
        ):
            qkvw_sb = consts.tile([128, 6, CCH, 384], bf16)
            projw_sb = consts.tile([128, CCH, C], bf16)
            bias_sb = consts.tile([1, C], f32)
            bias_bc = consts.tile([128, C], f32)
            # per-head zero-padded K tiles (parity-alternating): head i of a
            # pair occupies partition rows [64i, 64i+64); the other 64 rows are
            # zero so S-matmuls contract over the full 128 partitions with the
            # full Q tile as rhs (avoids K=64 bf16 matmuls, which miscompile)
            kpad = [
                consts.tile([128, 6, 2, 2, NTOK], bf16, name=f"kpad{p}")
                for p in range(2)
            ]
            # the very first weight chunk gets its own tiny tile so the first
            # ph1 unit depends on a 546ns transfer instead of the whole piece
            qm0_sb = consts.tile([128, CCH, 128], bf16)


            # ---- startup DMAs ----
            # DMA engines are a serial resource; order transfers by first use
            # (Q first half, xt stream 0, Q second half, xt stream 1, K, V,
            # proj, bias) and spread them over queues so the issue pipeline
            # keeps the device fed.
            xt_tiles = [None] * SAMPLES

            def fetch_xt(s, st, eng):
                if xt_tiles[s] is None:
                    xt_tiles[s] = xtp.tile(
                        [128, 2, CCH, NTOK], bf16, tag="xt", name=f"xt_{s}"
                    )
                eng.dma_start(
                    out=xt_tiles[s][:, st, :, :],
                    in_=xt_d[2 * s + st].rearrange("(c p) n -> p c n", p=128),
                )

            def fetch_w(piece, eng):
                eng.dma_start(
                    out=qkvw_sb[:, piece, :, :],
                    in_=qkvw_v[:, :, piece * 384 : (piece + 1) * 384],
                )

            nc.sync.dma_start(out=qm0_sb, in_=qkvw_v[:, :, 0:128])
            fetch_xt(0, 0, nc.scalar)
            nc.sync.dma_start(
                out=qkvw_sb[:, 0, :, 128:384], in_=qkvw_v[:, :, 128:384]
            )
            fetch_w(1, nc.scalar)  # Q second half
            fetch_xt(0, 1, nc.gpsimd)
            fetch_w(2, nc.sync)  # K first half
            fetch_w(3, nc.sync)  # K second half
            fetch_w(4, nc.sync)  # V first half
            fetch_w(5, nc.sync)  # V second half
            nc.sync.dma_start(out=projw_sb, in_=projw_v)
            nc.sync.dma_start(out=bias_sb, in_=bias_d[:, :])
            nc.gpsimd.partition_broadcast(bias_bc, bias_sb)
            for p in range(2):
                nc.gpsimd.memset(kpad[p][64:128, :, 0, :, :], 0.0)
                nc.gpsimd.memset(kpad[p][0:64, :, 1, :, :], 0.0)

            # ---- per-sample tiles ----
            qkt_tiles = [None] * SAMPLES
            v1_tiles = [None] * SAMPLES
            ot_tiles = {}

            # ---- filler machinery ----
            from collections import deque

            filler = deque()  # (kind, closure); kind "p12" must flush at sample boundary

            def drain(k):
                for _ in range(min(k, len(filler))):
                    filler.popleft()[1]()

            def drain_p12(s):
                # flush everything up to and including the last ph1/ph2 unit of
                # sample s; later ph4 units stay queued as attention filler
                while any(kind == ("p12", s) for kind, _ in filler):
                    filler.popleft()[1]()

            def ph1_unit(s, m, st):
                def emit():
                    eng_copy = (
                        nc.vector.tensor_copy
                        if (m + st) % 2 == 0
                        else (lambda out, in_: nc.scalar.activation(out, in_, Act.Copy))
                    )
                    pa = pap.tile([128, NTOK], f32, tag="pa", name=f"p1_{s}_{m}_{st}")
                    for c in range(CCH):
                        lhsT = (
                            qm0_sb[:, c, :]
                            if m == 0
                            else qkvw_sb[:, m // 3, c, (m % 3) * 128 : (m % 3 + 1) * 128]
                        )
                        nc.tensor.matmul(
                            pa,
                            lhsT,
                            xt_tiles[s][:, st, c, :],
                            start=(c == 0),
                            stop=(c == CCH - 1),
                        )
                    if m < 6:
                        eng_copy(out=qkt_tiles[s][:, m, st, :], in_=pa)
                    else:
                        kp = kpad[s % 2]
                        nc.vector.tensor_copy(
                            out=kp[0:64, m - 6, 0, st, :], in_=pa[0:64, :]
                        )
                        nc.scalar.activation(
                            kp[64:128, m - 6, 1, st, :], pa[64:128, :], Act.Copy
                        )

                return emit

            def ph2_unit(s, t, st, n):
                def emit():
                    pa = pap.tile([128, NTOK], f32, tag="pa", name=f"p2_{s}_{t}_{st}_{n}")
                    for c in range(CCH):
                        nc.tensor.matmul(
                            pa,
                            xt_tiles[s][:, st, c, t * 128 : (t + 1) * 128],
                            qkvw_sb[:, 4 + n, c, :],
                            start=(c == 0),
                            stop=(c == CCH - 1),
                        )
                    if (t + st + n) % 2 == 0:
                        nc.vector.tensor_copy(
                            out=v1_tiles[s][:, t, st, 6 * n : 6 * n + 6, 0:64],
                            in_=pa.rearrange("p (h d) -> p h d", h=6),
                        )
                    else:
                        nc.scalar.activation(
                            v1_tiles[s][:, t, st, 6 * n : 6 * n + 6, 0:64],
                            pa.rearrange("p (h d) -> p h d", h=6),
                            Act.Copy,
                        )

                return emit

            y_tiles = {}

            def ph4_unit(s, st, t, n2):
                def emit():
                    pa = pap.tile([128, NTOK], f32, tag="pa", name=f"p4_{s}_{st}_{t}_{n2}")
                    ot = ot_tiles[(s, st)]
                    for c in range(CCH):
                        nc.tensor.matmul(
                            pa,
                            ot[:, c, t * 128 : (t + 1) * 128],
                            projw_sb[:, c, n2 * NTOK : (n2 + 1) * NTOK],
                            start=(c == 0),
                            stop=(c == CCH - 1),
                        )
                    if n2 == 0:
                        y_tiles[(s, st, t)] = yp.tile(
                            [128, C], f32, tag="y", name=f"y_{s}_{st}_{t}"
                        )
                    y_sb = y_tiles[(s, st, t)]
                    nc.vector.tensor_add(
                        out=y_sb[:, n2 * NTOK : (n2 + 1) * NTOK],
                        in0=pa,
                        in1=bias_bc[:, n2 * NTOK : (n2 + 1) * NTOK],
                    )
                    if n2 == 1:
                        nc.sync.dma_start(
                            out=y_d[2 * s + st, t * 128 : (t + 1) * 128, :], in_=y_sb
                        )

                return emit

            def enqueue_ph12(s):
                qkt_tiles[s] = qktp.tile(
                    [128, 6, 2, NTOK], bf16, tag="qkt", name=f"qkt_{s}"
                )
                v1_tiles[s] = v1p.tile(
                    [128, TCH, 2, H, 65], bf16, tag="v1", name=f"v1_{s}"
                )
                k = ("p12", s)
                for st in range(2):
                    for m in range(MCH):
                        filler.append((k, ph1_unit(s, m, st)))
                # ones column for the softmax denominator
                filler.append(
                    (k, lambda s=s: nc.vector.memset(v1_tiles[s][:, :, :, :, 64:65], 1.0))
                )
                for t in range(TCH):
                    for st in range(2):
                        for n in range(2):
                            filler.append((k, ph2_unit(s, t, st, n)))

            def enqueue_ph4(s, st):
                for t in range(TCH):
                    for n2 in range(2):
                        filler.append((("p4", s), ph4_unit(s, st, t, n2)))

            def emit_s0e0(s, st, hp):
                # own-template keys x all queries (template + search), plus the
                # exp — computed one pair AHEAD so the exp latency is off the
                # critical path of the consuming pair's first AV matmul
                qkt = qkt_tiles[s]
                kp = kpad[s % 2]
                qT = qkt[:, hp, st, :]
                es0 = []
                for i in range(2):
                    p = scSp.tile(
                        [128, NTOK], f32, tag="sS", name=f"s0_{s}_{st}_{hp}_{i}"
                    )
                    nc.tensor.matmul(
                        p, kp[:, hp, i, st, 0:MT], qT, start=True, stop=True
                    )
                    e = es0p.tile(
                        [128, NTOK], bf16, tag="e0", name=f"e0_{s}_{st}_{hp}_{i}"
                    )
                    nc.scalar.activation(e, p, Act.Exp, scale=SCALE)
                    es0.append(e)
                return es0

            def attention_pair(s, st, hp, es0, nxt, budget):
                qkt = qkt_tiles[s]
                kp = kpad[s % 2]
                v1 = v1_tiles[s]
                ot = ot_tiles[(s, st)]
                qT = qkt[:, hp, st, :]
                po = []

                def dr():
                    nonlocal budget
                    if budget > 0 and filler:
                        budget -= 1
                        filler.popleft()[1]()

                # cross-modal template keys x search queries
                sc1 = scSp.tile([128, 2, 256], f32, tag="sS", name=f"s1_{s}_{st}_{hp}")
                for i in range(2):
                    nc.tensor.matmul(
                        sc1[:, i, :], kp[:, hp, i, 1 - st, 0:MT], qT[:, MT:],
                        start=True, stop=True,
                    )
                dr()
                es1 = esSp.tile([128, 2, 256], bf16, tag="eS", name=f"e1_{s}_{st}_{hp}")
                nc.scalar.activation(es1, sc1, Act.Exp, scale=SCALE)
                # own-search keys chunk 0 x search queries
                sc2 = scSp.tile([128, 2, 256], f32, tag="sS", name=f"s2_{s}_{st}_{hp}")
                for i in range(2):
                    nc.tensor.matmul(
                        sc2[:, i, :], kp[:, hp, i, st, MT : MT + 128], qT[:, MT:],
                        start=True, stop=True,
                    )
                dr()
                es2 = esSp.tile([128, 2, 256], bf16, tag="eS", name=f"e2_{s}_{st}_{hp}")
                nc.scalar.activation(es2, sc2, Act.Exp, scale=SCALE)
                # own-search keys chunk 1 x search queries
                sc3 = scSp.tile([128, 2, 256], f32, tag="sS", name=f"s3_{s}_{st}_{hp}")
                for i in range(2):
                    nc.tensor.matmul(
                        sc3[:, i, :], kp[:, hp, i, st, MT + 128 :], qT[:, MT:],
                        start=True, stop=True,
                    )
                dr()
                es3 = esSp.tile([128, 2, 256], bf16, tag="eS", name=f"e3_{s}_{st}_{hp}")
                nc.scalar.activation(es3, sc3, Act.Exp, scale=SCALE)
                # AV accumulation (template AV uses the pipelined es0)
                for i in range(2):
                    h = 2 * hp + i
                    p = pop.tile([65, NTOK], f32, tag="po", name=f"po_{s}_{st}_{hp}_{i}")
                    po.append(p)
                    nc.tensor.matmul(
                        p, v1[:, 0, st, h, :], es0[i], start=True, stop=False
                    )
                for i in range(2):
                    h = 2 * hp + i
                    nc.tensor.matmul(
                        po[i][:, MT:], v1[:, 0, 1 - st, h, :], es1[:, i, :],
                        start=False, stop=False,
                    )
                # next pair's template scores + exp, off the critical path
                nxt_es0 = emit_s0e0(s, *nxt) if nxt is not None else None
                dr()
                for i in range(2):
                    h = 2 * hp + i
                    nc.tensor.matmul(
                        po[i][:, MT:], v1[:, 1, st, h, :], es2[:, i, :],
                        start=False, stop=False,
                    )
                for i in range(2):
                    h = 2 * hp + i
                    nc.tensor.matmul(
                        po[i][:, MT:], v1[:, 2, st, h, :], es3[:, i, :],
                        start=False, stop=True,
                    )
                # norm: recips first, then broadcasts, then muls — keeps the
                # in-order DVE queue from head-of-line blocking on bcast0
                rls, rlbs = [], []
                for i in range(2):
                    rl = rlp.tile([1, NTOK], f32, tag="rl", name=f"rl_{s}_{st}_{hp}_{i}")
                    nc.vector.reciprocal(out=rl, in_=po[i][64:65, :])
                    rls.append(rl)
                for i in range(2):
                    rlb = rlbp.tile(
                        [64, NTOK], f32, tag="rlb", name=f"rlb_{s}_{st}_{hp}_{i}"
                    )
                    nc.gpsimd.partition_broadcast(rlb, rls[i])
                    rlbs.append(rlb)
                for i in range(2):
                    ro = i * 64
                    nc.vector.tensor_mul(ot[ro : ro + 64, hp, :], po[i][0:64, :], rlbs[i])
                return nxt_es0

            # ---- emission ----
            if SAMPLES > 1:
                fetch_xt(1, 0, nc.gpsimd)
                fetch_xt(1, 1, nc.gpsimd)
            enqueue_ph12(0)
            drain(len(filler))  # sample 0 ph1+ph2 run solid at startup
            for s in range(SAMPLES):
                if s + 2 < SAMPLES:
                    fetch_xt(s + 2, 0, nc.gpsimd)
                    fetch_xt(s + 2, 1, nc.gpsimd)
                if s + 1 < SAMPLES:
                    enqueue_ph12(s + 1)
                for st in range(2):
                    ot_tiles[(s, st)] = otp.tile(
                        [128, CCH, NTOK], bf16, tag="ot", name=f"ot_{s}_{st}"
                    )
                pairs = [(st, hp) for st in range(2) for hp in range(6)]
                es0 = emit_s0e0(s, 0, 0)
                for idx, (st, hp) in enumerate(pairs):
                    nxt = pairs[idx + 1] if idx + 1 < len(pairs) else None
                    budget = min(3, max(1, len(filler) // max(1, len(pairs) - idx)))
                    es0 = attention_pair(s, st, hp, es0, nxt, budget)
                    if (st, hp) == (0, 5):
                        enqueue_ph4(s, 0)
                enqueue_ph4(s, 1)
                if s + 1 < SAMPLES:
                    # sample s+1's ph1/ph2 must all be emitted before its
                    # attention starts; ph4 units stay queued as filler
                    drain_p12(s + 1)
            drain(len(filler))

    _lp.__exit__(None, None, None)
    nc.compile()
    return nc


def _get_program(**_ignored):
    if "prog" not in _PROG_CACHE:
        _PROG_CACHE["prog"] = _build_program()
    return _PROG_CACHE["prog"]


def _prep_in_maps(x_v, x_i, qkv_w, proj_w, proj_b):
    import ml_dtypes

    bf16 = ml_dtypes.bfloat16
    qkvwT = np.ascontiguousarray(np.asarray(qkv_w, np.float32).T).astype(bf16)
    projwT = np.ascontiguousarray(np.asarray(proj_w, np.float32).T).astype(bf16)
    bias = np.ascontiguousarray(np.asarray(proj_b, np.float32).reshape(1, C))
    in_maps = []
    for core in range(N_CORES):
        sl = slice(core * SAMPLES, (core + 1) * SAMPLES)
        # interleave: stream 2s = v-sample, 2s+1 = i-sample, transposed to [C, NTOK]
        xs = np.empty((2 * SAMPLES, C, NTOK), np.float32)
        xs[0::2] = np.asarray(x_v[sl], np.float32).transpose(0, 2, 1)
        xs[1::2] = np.asarray(x_i[sl], np.float32).transpose(0, 2, 1)
        in_maps.append(
            {
                "xt": xs.astype(bf16),
                "qkvwT": qkvwT,
                "projwT": projwT,
                "bias": bias,
            }
        )
    return in_maps


def kernel(x_v, x_i, qkv_w, proj_w, proj_b, t_h, t_w, s_h, s_w, num_heads):
    from concourse.bass_utils import run_bass_kernel_spmd

    nc = _get_program()
    in_maps = _prep_in_maps(x_v, x_i, qkv_w, proj_w, proj_b)
    res = run_bass_kernel_spmd(nc, in_maps, list(range(N_CORES)))
    out_v = np.empty((B, NTOK, C), np.float32)
    out_i = np.empty((B, NTOK, C), np.float32)
    for core in range(N_CORES):
        y = res.results[core]["y"]
        sl = slice(core * SAMPLES, (core + 1) * SAMPLES)
        out_v[sl] = y[0::2]
        out_i[sl] = y[1::2]
    return out_v, out_i
